# revision 1
# baseline (speedup 1.0000x reference)
"""CanonCausalMultiheadAttn Trainium2 kernel.

Sharding: 8 cores = 2 (batch) x 4 (kv-head groups). Core c handles batch
c//4 and kv-group g=c%4 (q heads 4g..4g+3, kv head g). w_q/w_k/w_v are
column-sharded by head group, w_o row-sharded; each core emits a partial
[S, D] output which the host sums over the 4 groups of its batch.

Per-core dataflow (everything in transposed [feature, token] layout so no
on-chip transposes are needed except v):
  qkvT[f, t] = w_qkv[:, f].T @ hT[:, t]          (bf16 matmuls, fp32 PSUM)
  conv: per-partition-scalar taps along the free (token) axis + residual
  scores.T[k, q] = kT.T @ qT  -> exp (no max-sub needed; |s|<~8) -> probsT
  causal: only k-tiles with k0 <= q_end computed; diagonal tiles use a
  precomputed multiplicative triangular mask and a shrunken q-region.
  attT[dh, q] += v_nat[k,:].T @ probsT   (v_nat from 16 PE transposes)
  sums[*, q]  += ones128.T @ probsT      (softmax denominator, replicated
                                          across partitions so DVE can divide)
  out[t, d]  = attT.T @ w_o_rows         (partial; host sums over groups)
"""

import numpy as np
import ml_dtypes
from contextlib import ExitStack

import concourse.bass as bass
import concourse.tile as tile
import concourse.mybir as mybir
from concourse.bass import ds, ts
from concourse.bass_utils import run_bass_kernel_spmd
from concourse.masks import make_identity

BF16 = mybir.dt.bfloat16
F32 = mybir.dt.float32
P = 128
S = 2048          # sequence length
D = 2048          # d_model
NF = 6            # feature chunks of 128: 4 q heads, 1 k, 1 v
KT = D // P       # 16 contraction chunks over d_model
NQT = S // 512    # 4 query tiles of 512
NTT = S // 512    # 4 token tiles of 512
ISQ = 1.0 / np.sqrt(128.0)
MULT = mybir.AluOpType.mult
ADD = mybir.AluOpType.add

_CACHE = {}


def _legalize_waits(nc):
    """Split multi-wait sync_info into preceding single-wait engine NOPs.

    The walrus codegen in this container accepts at most ONE sync wait per
    TPB instruction ("Too many sync wait commands"), but the Tile scheduler
    freely emits several. An engine executes its queue in order, so hoisting
    the extra waits onto NoOps right before the instruction is equivalent.
    """
    n = 0
    for f in nc.m.functions:
        for blk in f.blocks:
            out = []
            changed = False
            for inst in blk.instructions:
                si = inst.sync_info
                if (si is not None and si.on_wait and len(si.on_wait) > 1
                        and str(inst.engine) != "EngineType.Unassigned"):
                    waits = list(si.on_wait)
                    for w in waits[:-1]:
                        out.append(mybir.InstNoOp(
                            name=f"I-wf{n}", engine=inst.engine, ins=[],
                            outs=[],
                            sync_info=mybir.SyncInfo(on_wait=[w],
                                                     on_update=[])))
                        n += 1
                    si.on_wait = [waits[-1]]
                    changed = True
                out.append(inst)
            if changed:
                blk.instructions = out
    return n


def _build():
    if "nc" in _CACHE:
        return _CACHE["nc"]
    nc = bass.Bass("TRN2", target_bir_lowering=False, debug=False)

    hT_d = nc.dram_tensor("hT", [D, S], BF16, kind="ExternalInput").ap()
    wqkv_d = nc.dram_tensor("w_qkv", [D, NF * P], BF16, kind="ExternalInput").ap()
    wo_d = nc.dram_tensor("w_o", [4 * P, D], BF16, kind="ExternalInput").ap()
    cw_d = nc.dram_tensor("conv_w", [P, NF * 4], F32, kind="ExternalInput").ap()
    out_d = nc.dram_tensor("out", [S, D], F32, kind="ExternalOutput").ap()

    hT_v = hT_d.rearrange("(ko ki) t -> ki ko t", ki=P)        # [128,16,2048]
    wqkv_v = wqkv_d.rearrange("(ko ki) f -> ki ko f", ki=P)    # [128,16,768]
    wo_v = wo_d.rearrange("(c ki) d -> ki c d", ki=P)          # [128,4,2048]
    out_v = out_d.rearrange("(po pi) d -> pi po d", pi=P)      # [128,16,2048]

    with tile.TileContext(nc) as tc, ExitStack() as ctx:
        const = ctx.enter_context(tc.tile_pool(name="const", bufs=1))
        p_ht = ctx.enter_context(tc.tile_pool(name="ht", bufs=2))
        p_work = ctx.enter_context(tc.tile_pool(name="work", bufs=2))
        p_probs = ctx.enter_context(tc.tile_pool(name="probs", bufs=3))
        p_out = ctx.enter_context(tc.tile_pool(name="outp", bufs=6))
        ps2 = ctx.enter_context(tc.tile_pool(name="ps2", bufs=2, space="PSUM"))
        ps3 = ctx.enter_context(tc.tile_pool(name="ps3", bufs=3, space="PSUM"))
        ps1 = ctx.enter_context(tc.tile_pool(name="ps1", bufs=1, space="PSUM"))

        # --- constants / persistent tensors ---
        ident = const.tile([P, P], BF16, tag="ident")
        make_identity(nc, ident)
        # mask[k, x] = 1.0 if x >= k else 0.0 (shared by all diagonal tiles)
        mask = const.tile([P, 512], BF16, tag="mask")
        nc.gpsimd.memset(mask, 1.0)
        nc.gpsimd.affine_select(
            out=mask, in_=mask, pattern=[[1, 512]], base=0,
            channel_multiplier=-1, compare_op=mybir.AluOpType.is_ge, fill=0.0,
        )
        cw0 = const.tile([P, NF * 4], F32, tag="cw0")
        nc.sync.dma_start(cw0, cw_d)
        # conv ops read cw via a DVE copy so their DMA wait lands here, not
        # on the (wait-slot-limited) scalar_tensor_tensor instructions
        cw = const.tile([P, NF * 4], F32, tag="cw")
        nc.vector.tensor_copy(cw, cw0)
        wq_sb = const.tile([P, KT, NF * P], BF16, tag="wq")
        wo_sb = const.tile([P, 4, D], BF16, tag="wo")
        # raw (pre-conv) qkv.T in fp32, with 3 leading zero columns so the
        # causal conv taps can read t-3..t-1 without edge cases
        qkvf = const.tile([P, NF, S + 4], F32, tag="qkvf")
        # zero the pad on ACT so conv's read-waits coalesce with the ACT
        # projection copies (one sem instead of two)
        nc.scalar.memzero(qkvf[:, :, 0:4])
        qkvb = const.tile([P, NF, S], BF16, tag="qkvb")    # conv'd qkv.T (bf16)
        vnat = const.tile([P, KT, P], BF16, tag="vnat")    # v in [token, dh]
        attT = const.tile([P, 4, S], BF16, tag="attT")     # attended.T per head

        FP32R = mybir.dt.float32r
        ones_f = const.tile([P, P], F32, tag="ones_f")
        nc.vector.memset(ones_f, 1.0)
        ones_r = const.tile([P, P], FP32R, tag="ones_r")
        nc.vector.tensor_copy(ones_r, ones_f)

        def o_proj_chunk(qt, t4, tags=("proj",)):
            # output projection for one token-128-tile of q-tile qt
            tt16 = qt * 4 + t4
            for dt in range(4):
                op = ps2.tile([P, 512], F32, tag=tags[dt % len(tags)])
                for fc4 in range(4):
                    nc.tensor.matmul(
                        op, lhsT=attT[:, fc4, ds(tt16 * P, P)],
                        rhs=wo_sb[:, fc4, ds(dt * 512, 512)],
                        start=(fc4 == 0), stop=(fc4 == 3))
                ob = p_out.tile([P, 512], F32, tag="ob")
                nc.scalar.copy(ob, op)
                nc.sync.dma_start(out_v[:, tt16, ds(dt * 512, 512)], ob)

        def attn_B(qt):
            # attention for q-tile qt (needs phase A token tiles <= qt);
            # the previous q-tile's output projection is interleaved per-head
            # as PE filler while ACT/DVE work through exp/softmax chains.
            for h in range(4):
                nk = 4 * (qt + 1)
                att = ps3.tile([P, 512], F32, tag="att")
                colsum = p_work.tile([P, 512], FP32R, tag="colsum")
                prev = None
                prev2 = None
                pr_quad = None
                for kt in range(nk + 2):
                    if kt == min(4, nk - 2) and qt > 0:
                        # previous q-tile's output projection emitted mid-head:
                        # PE filler while ACT/DVE chew exp + softmax chains
                        o_proj_chunk(qt - 1, h)
                    if kt < nk:
                        j = kt - 4 * qt
                        x0 = j * P if j >= 0 else 0
                        F = 512 - x0
                        sp = ps2.tile([P, 512], F32, tag="s")
                        nc.tensor.matmul(
                            sp[:, x0:512],
                            lhsT=qkvb[:, 4, ds(kt * P, P)],
                            rhs=qkvb[:, h, ds(qt * 512 + x0, F)],
                            start=True, stop=True,
                        )
                        if kt % 4 == 0:
                            pr_quad = p_probs.tile([P, 4, 512], BF16,
                                                   tag="probs")
                        pr = pr_quad[:, kt % 4, :]
                        nc.scalar.activation(
                            pr[:, x0:512], sp[:, x0:512],
                            mybir.ActivationFunctionType.Exp, scale=ISQ)
                        if j >= 0:
                            nc.vector.tensor_mul(
                                pr[:, x0:512], pr[:, x0:512], mask[:, 0:F])
                        cur = (pr, x0, kt)
                    else:
                        cur = None
                    if prev2 is not None:
                        ppr, px0, pkt = prev2
                        nc.tensor.matmul(
                            att[:, px0:512], lhsT=vnat[:, pkt, :],
                            rhs=ppr[:, px0:512],
                            start=(pkt == 0), stop=(pkt == nk - 1))
                        # softmax denominator: accumulate exp'd probs on DVE
                        # (partition dim reduced by ONE ones-matmul at the end)
                        if pkt == 0:
                            nc.vector.tensor_copy(colsum, ppr)
                        else:
                            nc.vector.tensor_add(
                                colsum[:, px0:512], colsum[:, px0:512],
                                ppr[:, px0:512])
                    prev2 = prev
                    prev = cur
                smp = ps1.tile([P, 512], F32, tag="small")
                nc.tensor.matmul(smp, lhsT=ones_r, rhs=colsum,
                                 start=True, stop=True)
                rec = p_work.tile([P, 512], F32, tag="rec")
                nc.vector.reciprocal(rec, smp)
                nc.vector.tensor_mul(attT[:, h, ts(qt, 512)], att, rec)

        # ------- Fused phases: per token tile: projection+conv, then the
        # attention q-tile that just became computable, then the (pipelined)
        # output projection of the previous q-tile. Keeps PE dense while
        # spreading ACT(exp)/DVE(softmax) work across the whole timeline.
        for tt in range(NTT):
            ht = p_ht.tile([P, KT, 512], BF16, tag="ht")
            for k2 in range(8):
                # pair-chunk DMAs: fewer dispatches than per-chunk, still
                # fine-grained enough that the first matmuls start early
                if tt == 0:
                    nc.sync.dma_start(wq_sb[:, ds(k2 * 2, 2), :],
                                      wqkv_v[:, ds(k2 * 2, 2), :])
                nc.sync.dma_start(ht[:, ds(k2 * 2, 2), :],
                                  hT_v[:, ds(k2 * 2, 2), ts(tt, 512)])
            t0 = tt * 512

            def conv(fc):
                # conv taps: out[t] = x[t] + sum_k x[t+k-3]*w[k]
                tmp = p_work.tile([P, 512], F32, tag="ctmp", name="ctmp")
                nc.vector.scalar_tensor_tensor(
                    tmp, qkvf[:, fc, ds(t0 + 0, 512)],
                    cw[:, fc * 4 + 0: fc * 4 + 1],
                    qkvf[:, fc, ds(t0 + 3, 512)], op0=MULT, op1=ADD)
                nc.vector.scalar_tensor_tensor(
                    tmp, qkvf[:, fc, ds(t0 + 1, 512)],
                    cw[:, fc * 4 + 1: fc * 4 + 2], tmp, op0=MULT, op1=ADD)
                nc.vector.scalar_tensor_tensor(
                    tmp, qkvf[:, fc, ds(t0 + 2, 512)],
                    cw[:, fc * 4 + 2: fc * 4 + 3], tmp, op0=MULT, op1=ADD)
                nc.vector.scalar_tensor_tensor(
                    qkvb[:, fc, ts(tt, 512)], qkvf[:, fc, ds(t0 + 3, 512)],
                    cw[:, fc * 4 + 3: fc * 4 + 4], tmp, op0=MULT, op1=ADD)

            if tt == 0:
                # split each fc group into two 8-chunk halves, all A-halves
                # first: unblocks 48 matmuls once half the head DMA burst has
                # landed instead of stalling on the last chunk
                for fc in range(NF):
                    pp = ps2.tile([P, 512], F32, tag="proj", name="pp")
                    for kk in range(8):
                        nc.tensor.matmul(
                            pp, lhsT=wq_sb[:, kk, ds(fc * P, P)],
                            rhs=ht[:, kk, :],
                            start=(kk == 0), stop=(kk == 7))
                    nc.scalar.copy(qkvf[:, fc, ds(3, 512)], pp)
                for fc in range(NF):
                    pp = ps2.tile([P, 512], F32, tag="proj", name="pp")
                    for kk in range(8, KT):
                        nc.tensor.matmul(
                            pp, lhsT=wq_sb[:, kk, ds(fc * P, P)],
                            rhs=ht[:, kk, :],
                            start=(kk == 8), stop=(kk == KT - 1))
                    nc.vector.tensor_add(
                        qkvf[:, fc, ds(3, 512)], qkvf[:, fc, ds(3, 512)], pp)
                    conv(fc)
            else:
                for fc in range(NF):
                    pp = ps2.tile([P, 512], F32, tag="proj", name="pp")
                    for kk in range(KT):
                        nc.tensor.matmul(
                            pp, lhsT=wq_sb[:, kk, ds(fc * P, P)],
                            rhs=ht[:, kk, :],
                            start=(kk == 0), stop=(kk == KT - 1),
                        )
                    nc.scalar.copy(qkvf[:, fc, ds(3 + t0, 512)], pp)
                    conv(fc)
            # v (fc=5) of this token tile -> natural [token, dh] layout
            for j in range(4):
                kt_i = tt * 4 + j
                trp = ps1.tile([P, 512], BF16, tag="small")
                nc.tensor.transpose(trp[:, 0:P], qkvb[:, 5, ds(kt_i * P, P)],
                                    ident)
                nc.vector.tensor_copy(vnat[:, kt_i, :], trp[:, 0:P])
            if tt == 0:
                # w_o load deferred past the critical head DMAs
                nc.sync.dma_start(wo_sb, wo_v)
            attn_B(tt)
        for t4 in range(4):
            o_proj_chunk(NQT - 1, t4, tags=("proj", "s"))

    _legalize_waits(nc)
    _CACHE["nc"] = nc
    return nc


def _prep_inputs(hidden_states, w_q, w_k, w_v, w_o, conv_w):
    """Build the 8 per-core input maps (host-side shard + bf16 cast)."""
    bf = ml_dtypes.bfloat16
    in_maps = []
    for c in range(8):
        b, g = c // 4, c % 4
        hT = np.ascontiguousarray(hidden_states[b].T).astype(bf)
        wqkv = np.concatenate(
            [w_q[:, g * 512:(g + 1) * 512],
             w_k[:, g * 128:(g + 1) * 128],
             w_v[:, g * 128:(g + 1) * 128]], axis=1).astype(bf)
        wo = np.ascontiguousarray(w_o[g * 512:(g + 1) * 512, :]).astype(bf)
        cw = np.concatenate(
            [conv_w[g * 512:(g + 1) * 512],
             conv_w[2048 + g * 128: 2048 + (g + 1) * 128],
             conv_w[2560 + g * 128: 2560 + (g + 1) * 128]], axis=0)  # [768,4]
        cw = np.ascontiguousarray(
            cw.reshape(NF, P, 4).transpose(1, 0, 2).reshape(P, NF * 4)
        ).astype(np.float32)
        in_maps.append({"hT": hT, "w_qkv": wqkv, "w_o": wo, "conv_w": cw})
    return in_maps


def kernel(hidden_states, w_q, w_k, w_v, w_o, conv_w, _trace=False):
    nc = _build()
    in_maps = _prep_inputs(
        np.asarray(hidden_states, dtype=np.float32),
        np.asarray(w_q, dtype=np.float32),
        np.asarray(w_k, dtype=np.float32),
        np.asarray(w_v, dtype=np.float32),
        np.asarray(w_o, dtype=np.float32),
        np.asarray(conv_w, dtype=np.float32),
    )
    res = run_bass_kernel_spmd(nc, in_maps, core_ids=list(range(8)),
                               trace=_trace)
    outs = [r["out"] for r in res.results]
    full = np.empty((2, S, D), dtype=np.float32)
    for b in range(2):
        full[b] = outs[4 * b] + outs[4 * b + 1] + outs[4 * b + 2] + outs[4 * b + 3]
    if _trace:
        kernel.last_results = res
    return full



# revision 50
# speedup vs baseline: 1.0292x; 1.0292x over previous
"""CanonCausalMultiheadAttn Trainium2 kernel (v2).

Sharding: 8 cores = 2 (batch) x 4 (kv-head groups). Core c handles batch
c//4 and kv-group g=c%4 (q heads 4g..4g+3, kv head g). w_q/w_k/w_v are
column-sharded by head group, w_o row-sharded; each core emits a partial
[S, D] output (bf16) which the host sums over the 4 groups of its batch.

Per-core dataflow (transposed [feature, token] layout):
  qkvT[f, t] = w_qkv[:, f].T @ hT[:, t]          (bf16 matmuls, fp32 PSUM)
  conv: per-partition-scalar taps along the free (token) axis + residual
        (chains split across DVE and GPSIMD/Pool)
  scores.T[k, q] = kT.T @ qT  -> exp (paired over two k-tiles) -> probsT
  causal: only k-tiles with k0 <= q_end computed; diagonal tiles use a
  precomputed multiplicative triangular mask and a shrunken q-region.
  attT[dh, q] += v_nat[k,:].T @ probsT   (v_nat via XBAR dma transposes)
  colsum (softmax denominator) accumulated in bf16 on DVE, partition-
  reduced with one ones-matmul per (head, q-tile)
  out[t, d]  = attT.T @ w_o_rows  (PSUM -> bf16 SBUF on Pool -> DMA)
"""

import numpy as np
import ml_dtypes
from contextlib import ExitStack

import concourse.bass as bass
import concourse.tile as tile
import concourse.mybir as mybir
from concourse.bass import ds, ts
from concourse.bass_utils import run_bass_kernel_spmd
from concourse.masks import make_identity

BF16 = mybir.dt.bfloat16
F32 = mybir.dt.float32
P = 128
S = 2048          # sequence length
D = 2048          # d_model
NF = 6            # feature chunks of 128: 4 q heads, 1 k, 1 v
KT = D // P       # 16 contraction chunks over d_model
NQT = S // 512    # 4 query tiles of 512
NTT = S // 512    # 4 token tiles of 512
ISQ = 1.0 / np.sqrt(128.0)
MULT = mybir.AluOpType.mult
ADD = mybir.AluOpType.add
EXP = mybir.ActivationFunctionType.Exp

_CACHE = {}


def _legalize_waits(nc):
    """Split multi-wait sync_info into preceding single-wait engine NOPs.

    The walrus codegen in this container accepts at most ONE sync wait per
    TPB instruction ("Too many sync wait commands"), but the Tile scheduler
    freely emits several. An engine executes its queue in order, so hoisting
    the extra waits onto NoOps right before the instruction is equivalent.
    """
    n = 0
    for f in nc.m.functions:
        for blk in f.blocks:
            out = []
            changed = False
            for inst in blk.instructions:
                si = inst.sync_info
                if (si is not None and si.on_wait and len(si.on_wait) > 1
                        and str(inst.engine) != "EngineType.Unassigned"):
                    waits = list(si.on_wait)
                    for w in waits[:-1]:
                        out.append(mybir.InstNoOp(
                            name=f"I-wf{n}", engine=inst.engine, ins=[],
                            outs=[],
                            sync_info=mybir.SyncInfo(on_wait=[w],
                                                     on_update=[])))
                        n += 1
                    si.on_wait = [waits[-1]]
                    changed = True
                out.append(inst)
            if changed:
                blk.instructions = out
    return n


def _build():
    if "nc" in _CACHE:
        return _CACHE["nc"]
    nc = bass.Bass("TRN2", target_bir_lowering=False, debug=False)

    hT_d = nc.dram_tensor("hT", [D, S], BF16, kind="ExternalInput").ap()
    wqkv_d = nc.dram_tensor("w_qkv", [D, NF * P], BF16, kind="ExternalInput").ap()
    wo_d = nc.dram_tensor("w_o", [4 * P, D], BF16, kind="ExternalInput").ap()
    cw_d = nc.dram_tensor("conv_w", [P, NF * 4], F32, kind="ExternalInput").ap()
    out_d = nc.dram_tensor("out", [S, D], BF16, kind="ExternalOutput").ap()

    hT_v = hT_d.rearrange("(ko ki) t -> ki ko t", ki=P)        # [128,16,2048]
    wqkv_v = wqkv_d.rearrange("(ko ki) f -> ki ko f", ki=P)    # [128,16,768]
    wo_v = wo_d.rearrange("(c ki) d -> ki c d", ki=P)          # [128,4,2048]
    out_v = out_d.rearrange("(po pi) d -> pi po d", pi=P)      # [128,16,2048]

    with tile.TileContext(nc) as tc, ExitStack() as ctx:
        const = ctx.enter_context(tc.tile_pool(name="const", bufs=1))
        p_ht = ctx.enter_context(tc.tile_pool(name="ht", bufs=2))
        p_work = ctx.enter_context(tc.tile_pool(name="work", bufs=2))
        p_probs = ctx.enter_context(tc.tile_pool(name="probs", bufs=3))
        p_ob = ctx.enter_context(tc.tile_pool(name="obp", bufs=3))
        ps_s = ctx.enter_context(tc.tile_pool(name="ps_s", bufs=2, space="PSUM"))
        ps_att = ctx.enter_context(tc.tile_pool(name="ps_att", bufs=1, space="PSUM"))
        ps_pr = ctx.enter_context(tc.tile_pool(name="ps_pr", bufs=2, space="PSUM"))

        # --- constants / persistent tensors ---
        # ones first: the PE warmup matmuls depend on it, and DVE executes
        # its queue in order
        ones_b = const.tile([P, P], BF16, tag="ones_b")
        nc.vector.memset(ones_b, 1.0)
        ident = const.tile([P, P], BF16, tag="ident")
        make_identity(nc, ident)
        # mask[k, x] = 1.0 if x >= k else 0.0 (shared by all diagonal tiles)
        mask = const.tile([P, 512], BF16, tag="mask")
        nc.gpsimd.memset(mask, 1.0)
        nc.gpsimd.affine_select(
            out=mask, in_=mask, pattern=[[1, 512]], base=0,
            channel_multiplier=-1, compare_op=mybir.AluOpType.is_ge, fill=0.0,
        )
        cw0 = const.tile([P, NF * 4], F32, tag="cw0")
        # (cw0's DMA and the cw/cwp copies are issued after the startup
        # wq/ht burst)
        cw = const.tile([P, NF * 4], F32, tag="cw")
        wq_sb = const.tile([P, KT, NF * P], BF16, tag="wq")
        wo_sb = const.tile([P, 4, D], BF16, tag="wo")
        # raw (pre-conv) qkv.T in fp32, with 3 leading zero columns so the
        # causal conv taps can read t-3..t-1 without edge cases
        qkvf = const.tile([P, NF, S + 4], F32, tag="qkvf")
        nc.scalar.memzero(qkvf[:, :, 0:4])
        qkvb = const.tile([P, NF, S], BF16, tag="qkvb")    # conv'd qkv.T (bf16)
        # v in [token, dh] with a ones column at 128: the PV matmul then
        # emits the softmax denominator as output column 128 for free
        # row stride 136 elements = 272 bytes: XBAR transpose destinations
        # must be 16-byte aligned (unaligned rows corrupt the transpose)
        vnat = const.tile([P, KT, 136], BF16, tag="vnat")
        nc.vector.memset(vnat[:, :, 128:129], 1.0)
        attT = const.tile([P, 4, S], BF16, tag="attT")     # attended.T per head

        def o_proj_chunk(qt, t4, dts=range(4), last=False):
            # output projection for token-128-tile t4 of q-tile qt
            tt16 = qt * 4 + t4
            ob = o_proj_chunk.ob
            if ob is None or o_proj_chunk.ob_tt != tt16:
                ob = p_ob.tile([P, 4, 512], BF16, tag="ob")
                o_proj_chunk.ob = ob
                o_proj_chunk.ob_tt = tt16
            for dt in dts:
                if last and (t4 * 4 + dt) % 2 == 1:
                    # the scores PSUM pool is free during the final chunks;
                    # alternating pools doubles the rotation depth
                    op = ps_s.tile([P, 512], F32, tag="s", name="op2")
                else:
                    op = ps_pr.tile([P, 512], F32, tag="proj")
                for fc4 in range(4):
                    nc.tensor.matmul(
                        op, lhsT=attT[:, fc4, ds(tt16 * P, P)],
                        rhs=wo_sb[:, fc4, ds(dt * 512, 512)],
                        start=(fc4 == 0), stop=(fc4 == 3))
                # PSUM -> bf16 SBUF copy: DVE during attention phases
                # (ACT is exp-bound); ACT in the final stretch where it is
                # otherwise idle
                if last:
                    nc.scalar.copy(ob[:, dt, :], op)
                else:
                    nc.vector.tensor_copy(ob[:, dt, :], op)
                if last:
                    nc.sync.dma_start(out_v[:, tt16, ds(dt * 512, 512)],
                                      ob[:, dt, :])
            if not last and dts[-1] == 3:
                nc.sync.dma_start(out_v[:, tt16, :], ob)

        o_proj_chunk.ob = None
        o_proj_chunk.ob_tt = -1

        fins = []

        def flush_fins():
            while fins:
                fins.pop(0)()

        def attn_head(qt, h):
            nk = 4 * (qt + 1)
            legacy = False
            lasthead = (qt == NQT - 1 and h == 3)
            # diagonal k-tiles first (their conv dependencies are a full
            # phase old under the staggered schedule), then full pairs
            units = [("diag", [4 * qt + j]) for j in range(4)]
            units += [("full", [kt]) for kt in range(4 * qt)]
            nu = len(units)
            # spread the previous q-tile's o_proj chunks over the units.
            # For qt==3 head 2 also takes head 3's share so the legacy-path
            # last head keeps a free ps_pr rotation slot.
            inj = {}
            if qt > 0:
                if qt < NQT - 1:
                    # the previous q-tile's h3 attT transposes land late in
                    # the previous pair; skip h0 so nothing stalls on them
                    q0 = qt - 1
                    work = {0: [], 1: [(0, 0), (0, 1), (0, 2), (0, 3)],
                            2: [(1, 0), (1, 1), (1, 2), (1, 3)],
                            3: [(2, 0), (2, 1), (2, 2), (2, 3),
                                (3, 0), (3, 1), (3, 2), (3, 3)]}[h]
                else:
                    work = [(h, 0), (h, 1), (h, 2), (h, 3)]
                if work:
                    step = (nu - 2) / len(work)
                    for i, w in enumerate(work):
                        inj.setdefault(2 + int(i * step), []).append(w)
            # one PSUM bank per q-subtile accumulator: concurrent matmul
            # accumulation groups must not share a bank (hardware
            # start/has_written tracking is bank-granular). Allocated lazily
            # at the first flush so the previous head's deferred fin tiles
            # rotate in front of them.
            sub_t = []

            def get_sub():
                if not sub_t:
                    sub_t.extend(
                        ps_att.tile([P, 129], F32, tag=f"att{s}", name="attx")
                        for s in range(4))
                return sub_t
            state = dict(prev=None, prev2=None, flushed=0, dn=0)
            scount = [0, 0, 0, 0]
            stotal = [4 * qt + s + 1 for s in range(4)]
            pr_quad = None

            def flush(unit):
                # PV matmuls for a finished unit. Non-legacy: per-128-q
                # subtile with a 129th output column accumulating the
                # softmax denominator. Legacy: [dh, q] with DVE colsum.
                for (pr, x0, kt) in unit:
                    for s in range(x0 // P, 4):
                        nc.tensor.matmul(
                            get_sub()[s], lhsT=pr[:, ds(s * P, P)],
                            rhs=vnat[:, kt, 0:129],
                            start=(scount[s] == 0),
                            stop=(scount[s] == stotal[s] - 1))
                        scount[s] += 1
                    state["flushed"] += 1

            for ui, (kind, kts) in enumerate(units):
                if ui == 1:
                    flush_fins()
                for (t4i, dti) in inj.get(ui, ()):
                    o_proj_chunk(qt - 1, t4i, dts=[dti])
                if kts[0] % 4 == 0:
                    pr_quad = p_probs.tile([P, 4, 512], BF16, tag="probs")
                kt = kts[0]
                if kind == "full":
                    sp = ps_s.tile([P, 512], F32, tag="s")
                    nc.tensor.matmul(
                        sp, lhsT=qkvb[:, 4, ds(kt * P, P)],
                        rhs=qkvb[:, h, ts(qt, 512)],
                        start=True, stop=True)
                    pr = pr_quad[:, kt % 4, :]
                    nc.scalar.activation(pr, sp, EXP, scale=ISQ)
                    cur = [(pr, 0, kt)]
                else:
                    j = kt - 4 * qt
                    x0 = j * P
                    F = 512 - x0
                    sp = ps_s.tile([P, 512], F32, tag="s")
                    nc.tensor.matmul(
                        sp[:, x0:512],
                        lhsT=qkvb[:, 4, ds(kt * P, P)],
                        rhs=qkvb[:, h, ds(qt * 512 + x0, F)],
                        start=True, stop=True)
                    pr = pr_quad[:, kt % 4, :]
                    nc.scalar.activation(
                        pr[:, x0:512], sp[:, x0:512], EXP, scale=ISQ)
                    nc.vector.tensor_mul(
                        pr[:, x0:512], pr[:, x0:512], mask[:, 0:F])
                    cur = [(pr, x0, kt)]
                if state["prev2"] is not None:
                    flush(state["prev2"])
                state["prev2"] = state["prev"]
                state["prev"] = cur
            if state["prev2"] is not None:
                flush(state["prev2"])
            if state["prev"] is not None:
                flush(state["prev"])

            # ---- finalize: reciprocal of the denominator columns, then
            # per-partition scale into bf16 [q, dh], then PE transpose into
            # attT. Phased: all scales, then all transposes, then all
            # copies (no DVE<->PE queue ping-pong). Deferred into the next
            # head unless this is the very last one.
            def fin():
                rec = p_work.tile([P, 4], F32, tag="rec4")
                for s in range(4):
                    nc.vector.reciprocal(rec[:, s:s + 1],
                                         sub_t[s][:, 128:129])
                anorms = []
                for s in range(4):
                    anorm = p_work.tile([P, P], BF16, tag=f"anorm{s}",
                                        name="anorm", bufs=3)
                    nc.vector.tensor_scalar_mul(
                        anorm, sub_t[s][:, 0:P], rec[:, s:s + 1])
                    anorms.append(anorm)
                tps = []
                for s in range(4):
                    tp = ps_att.tile([P, P], BF16, tag=f"att{s}", name="tp")
                    nc.tensor.transpose(tp, anorms[s], ident)
                    tps.append(tp)
                for s in range(4):
                    nc.vector.tensor_copy(
                        attT[:, h, ds(qt * 512 + s * P, P)], tps[s])
            if lasthead:
                fin()
            else:
                fins.append(fin)

        def attn_heads(qt):
            for h in range(4):
                attn_head(qt, h)
                yield

        def conv(fc, eng, cwt, etag):
            # conv taps: out[t] = x[t] + sum_k x[t+k-3]*w[k]
            t0 = conv.t0
            tmp = p_work.tile([P, 512], F32, tag="ctmp" + etag, name="ctmp")
            eng.scalar_tensor_tensor(
                tmp, qkvf[:, fc, ds(t0 + 0, 512)],
                cwt[:, fc * 4 + 0: fc * 4 + 1],
                qkvf[:, fc, ds(t0 + 3, 512)], op0=MULT, op1=ADD)
            eng.scalar_tensor_tensor(
                tmp, qkvf[:, fc, ds(t0 + 1, 512)],
                cwt[:, fc * 4 + 1: fc * 4 + 2], tmp, op0=MULT, op1=ADD)
            eng.scalar_tensor_tensor(
                tmp, qkvf[:, fc, ds(t0 + 2, 512)],
                cwt[:, fc * 4 + 2: fc * 4 + 3], tmp, op0=MULT, op1=ADD)
            eng.scalar_tensor_tensor(
                qkvb[:, fc, ds(t0, 512)], qkvf[:, fc, ds(t0 + 3, 512)],
                cwt[:, fc * 4 + 3: fc * 4 + 4], tmp, op0=MULT, op1=ADD)

        # conv engine assignment: q-head chains 0,1 on DVE (light during
        # projection), the rest on Pool
        CONV_ENG = {}

        def run_conv(fc):
            eng, cwt, etag = CONV_ENG[fc]
            conv(fc, eng, cwt, etag)

        # ------- Staggered pipeline: proj(tt) runs one phase AHEAD of
        # attn(tt-1), so attention never waits on freshly conv'd qkv and the
        # PE queue (strict FIFO) always has ready matmuls at phase edges.
        CONV_ENG.update({fc: (nc.vector, cw, "v") for fc in range(NF)})
        FC_ORDER = [0, 4, 5, 1, 2, 3]

        # warm up the PE p-state during the initial DMA wait with dummy
        # matmuls into a scratch PSUM tile (output never read)
        warm = ps_pr.tile([P, 512], F32, tag="proj", name="warm")
        for _ in range(8):
            nc.tensor.matmul(warm[:, 0:128], lhsT=ones_b, rhs=ones_b,
                             start=True, stop=True, skip_group_check=True)

        ht_cur = p_ht.tile([P, KT, 512], BF16, tag="ht")
        # fine-grained startup: single wq chunks + ht pieces ordered so the
        # first contraction chunks land first
        nc.sync.dma_start(ht_cur[:, 0:1, :], hT_v[:, 0:1, ts(0, 512)])
        nc.sync.dma_start(wq_sb[:, 0:1, 0:384], wqkv_v[:, 0:1, 0:384])
        nc.sync.dma_start(wq_sb[:, 0:1, 384:768], wqkv_v[:, 0:1, 384:768])
        nc.sync.dma_start(ht_cur[:, 1:2, :], hT_v[:, 1:2, ts(0, 512)])
        nc.sync.dma_start(wq_sb[:, 1:2, :], wqkv_v[:, 1:2, :])
        for k2 in range(1, 8):
            nc.sync.dma_start(ht_cur[:, ds(k2 * 2, 2), :],
                              hT_v[:, ds(k2 * 2, 2), ts(0, 512)])
            nc.sync.dma_start(wq_sb[:, ds(2 * k2, 2), :],
                              wqkv_v[:, ds(2 * k2, 2), :])
        nc.sync.dma_start(cw0, cw_d)
        # conv ops read cw via per-engine copies so their DMA wait lands
        # here, not on the (wait-slot-limited) scalar_tensor_tensor chains
        nc.vector.tensor_copy(cw, cw0)

        def emit_vnat(tt):
            # v (fc=5) of this token tile -> natural [token, dh] layout via
            # PE transpose (the XBAR transpose path raced under load)
            for j in range(4):
                kt_i = tt * 4 + j
                tv = ps_pr.tile([P, P], BF16, tag="proj", name="tv")
                nc.tensor.transpose(tv, qkvb[:, 5, ds(kt_i * P, P)], ident)
                nc.vector.tensor_copy(vnat[:, kt_i, 0:P], tv)

        def proj_blocks(tt, ht):
            # generator: yields after each fc block so attention heads of the
            # previous q-tile can interleave as PE filler
            conv.t0 = t0 = tt * 512
            if tt == 0:
                # first half contraction-outer (6 concurrent PSUM groups, so
                # matmuls start as soon as the first chunk pair lands); second
                # half fc-outer so each fc's copy+conv chain starts early
                gA = ps_s.tile([P, 512], F32, tag="s", name="gA")
                gB = ps_s.tile([P, 512], F32, tag="s", name="gB")
                gl = [ps_att.tile([P, 512], F32, tag=f"att{s}", name="gx")
                      for s in range(4)]
                groups = {0: gA, 4: gB, 5: gl[0],
                          1: gl[1], 2: gl[2], 3: gl[3]}
                for kk in range(KT // 2):
                    for fc in FC_ORDER:
                        nc.tensor.matmul(
                            groups[fc], lhsT=wq_sb[:, kk, ds(fc * P, P)],
                            rhs=ht[:, kk, :],
                            start=(kk == 0), stop=False)
                for fc in FC_ORDER:
                    for kk in range(KT // 2, KT):
                        nc.tensor.matmul(
                            groups[fc], lhsT=wq_sb[:, kk, ds(fc * P, P)],
                            rhs=ht[:, kk, :],
                            start=False, stop=(kk == KT - 1))
                    nc.scalar.copy(qkvf[:, fc, ds(3 + t0, 512)], groups[fc])
                    if fc in (4, 5):
                        run_conv(fc)
                    if fc == 5:
                        emit_vnat(tt)
                for fc in (0, 1, 2, 3):
                    run_conv(fc)
            else:
                for fc in FC_ORDER:
                    pp = ps_pr.tile([P, 512], F32, tag="proj", name="pp")
                    for kk in range(KT):
                        nc.tensor.matmul(
                            pp, lhsT=wq_sb[:, kk, ds(fc * P, P)],
                            rhs=ht[:, kk, :],
                            start=(kk == 0), stop=(kk == KT - 1),
                        )
                    nc.scalar.copy(qkvf[:, fc, ds(3 + t0, 512)], pp)
                    # k/v convs run inline (their consumers are earliest in
                    # the next phase); q convs are emitted at phase end so
                    # attention's DVE/Pool work isn't queued behind them
                    if fc in (4, 5):
                        run_conv(fc)
                    if fc == 5:
                        emit_vnat(tt)
                    yield
                for fc in (0, 1, 2, 3):
                    run_conv(fc)

        def drain(gen):
            if gen is not None:
                for _ in gen:
                    pass

        for tt in range(NTT):
            if tt + 1 < NTT:
                ht_next = p_ht.tile([P, KT, 512], BF16, tag="ht")
            else:
                ht_next = None
            prefetched = False

            def emit_prefetch(tt, ht_next):
                for k4 in range(4):
                    nc.sync.dma_start(
                        ht_next[:, ds(k4 * 4, 4), :],
                        hT_v[:, ds(k4 * 4, 4), ts(tt + 1, 512)])
                if tt == 0:
                    # w_o load deferred out of the startup DMA burst, but
                    # before the attention-phase transposes start competing
                    nc.sync.dma_start(wo_sb, wo_v)

            pg = proj_blocks(tt, ht_cur)
            ag = attn_heads(tt - 1) if tt >= 1 else None
            if tt == 1:
                # attn(0) depends on tt=0's conv chains, which retire during
                # proj(1); give proj one block's head start
                next(pg)
            # alternate: fc projection block, then attention head
            step = 0
            while True:
                stopped = True
                try:
                    next(pg)
                    stopped = False
                except StopIteration:
                    pass
                if ag is not None:
                    try:
                        next(ag)
                        stopped = False
                    except StopIteration:
                        ag = None
                step += 1
                if step == 4 and ht_next is not None:
                    # prefetch next token tile's hidden states mid-pair so
                    # the attention fins' attT transposes win HWDGE first
                    emit_prefetch(tt, ht_next)
                    prefetched = True
                if stopped and ag is None:
                    break
            if ht_next is not None and not prefetched:
                emit_prefetch(tt, ht_next)
            ht_cur = ht_next
        drain(attn_heads(NQT - 1))
        flush_fins()
        for t4 in range(4):
            o_proj_chunk(NQT - 1, t4, last=(t4 == 3))

    _legalize_waits(nc)
    _CACHE["nc"] = nc
    return nc


def _prep_inputs(hidden_states, w_q, w_k, w_v, w_o, conv_w):
    """Build the 8 per-core input maps (host-side shard + bf16 cast)."""
    bf = ml_dtypes.bfloat16
    in_maps = []
    for c in range(8):
        b, g = c // 4, c % 4
        hT = np.ascontiguousarray(hidden_states[b].T).astype(bf)
        wqkv = np.concatenate(
            [w_q[:, g * 512:(g + 1) * 512],
             w_k[:, g * 128:(g + 1) * 128],
             w_v[:, g * 128:(g + 1) * 128]], axis=1).astype(bf)
        wo = np.ascontiguousarray(w_o[g * 512:(g + 1) * 512, :]).astype(bf)
        cw = np.concatenate(
            [conv_w[g * 512:(g + 1) * 512],
             conv_w[2048 + g * 128: 2048 + (g + 1) * 128],
             conv_w[2560 + g * 128: 2560 + (g + 1) * 128]], axis=0)  # [768,4]
        cw = np.ascontiguousarray(
            cw.reshape(NF, P, 4).transpose(1, 0, 2).reshape(P, NF * 4)
        ).astype(np.float32)
        in_maps.append({"hT": hT, "w_qkv": wqkv, "w_o": wo, "conv_w": cw})
    return in_maps


def kernel(hidden_states, w_q, w_k, w_v, w_o, conv_w, _trace=False):
    nc = _build()
    in_maps = _prep_inputs(
        np.asarray(hidden_states, dtype=np.float32),
        np.asarray(w_q, dtype=np.float32),
        np.asarray(w_k, dtype=np.float32),
        np.asarray(w_v, dtype=np.float32),
        np.asarray(w_o, dtype=np.float32),
        np.asarray(conv_w, dtype=np.float32),
    )
    res = run_bass_kernel_spmd(nc, in_maps, core_ids=list(range(8)),
                               trace=_trace)
    outs = [np.asarray(r["out"], dtype=np.float32) for r in res.results]
    full = np.empty((2, S, D), dtype=np.float32)
    for b in range(2):
        full[b] = outs[4 * b] + outs[4 * b + 1] + outs[4 * b + 2] + outs[4 * b + 3]
    if _trace:
        kernel.last_results = res
    return full


# revision 62
# speedup vs baseline: 1.0295x; 1.0003x over previous
"""CanonCausalMultiheadAttn Trainium2 kernel (v2).

Sharding: 8 cores = 2 (batch) x 4 (kv-head groups). Core c handles batch
c//4 and kv-group g=c%4 (q heads 4g..4g+3, kv head g). w_q/w_k/w_v are
column-sharded by head group, w_o row-sharded; each core emits a partial
[S, D] output (bf16) which the host sums over the 4 groups of its batch.

Per-core dataflow (transposed [feature, token] layout):
  qkvT[f, t] = w_qkv[:, f].T @ hT[:, t]          (bf16 matmuls, fp32 PSUM)
  conv: per-partition-scalar taps along the free (token) axis + residual
        (chains split across DVE and GPSIMD/Pool)
  scores.T[k, q] = kT.T @ qT  -> exp (paired over two k-tiles) -> probsT
  causal: only k-tiles with k0 <= q_end computed; diagonal tiles use a
  precomputed multiplicative triangular mask and a shrunken q-region.
  attT[dh, q] += v_nat[k,:].T @ probsT   (v_nat via XBAR dma transposes)
  colsum (softmax denominator) accumulated in bf16 on DVE, partition-
  reduced with one ones-matmul per (head, q-tile)
  out[t, d]  = attT.T @ w_o_rows  (PSUM -> bf16 SBUF on Pool -> DMA)
"""

import numpy as np
import ml_dtypes
from contextlib import ExitStack

import concourse.bass as bass
import concourse.tile as tile
import concourse.mybir as mybir
from concourse.bass import ds, ts
from concourse.bass_utils import run_bass_kernel_spmd
from concourse.masks import make_identity

BF16 = mybir.dt.bfloat16
F32 = mybir.dt.float32
P = 128
S = 2048          # sequence length
D = 2048          # d_model
NF = 6            # feature chunks of 128: 4 q heads, 1 k, 1 v
KT = D // P       # 16 contraction chunks over d_model
NQT = S // 512    # 4 query tiles of 512
NTT = S // 512    # 4 token tiles of 512
ISQ = 1.0 / np.sqrt(128.0)
MULT = mybir.AluOpType.mult
ADD = mybir.AluOpType.add
EXP = mybir.ActivationFunctionType.Exp

_CACHE = {}


def _legalize_waits(nc):
    """Split multi-wait sync_info into preceding single-wait engine NOPs.

    The walrus codegen in this container accepts at most ONE sync wait per
    TPB instruction ("Too many sync wait commands"), but the Tile scheduler
    freely emits several. An engine executes its queue in order, so hoisting
    the extra waits onto NoOps right before the instruction is equivalent.
    """
    n = 0
    for f in nc.m.functions:
        for blk in f.blocks:
            out = []
            changed = False
            for inst in blk.instructions:
                si = inst.sync_info
                if (si is not None and si.on_wait and len(si.on_wait) > 1
                        and str(inst.engine) != "EngineType.Unassigned"):
                    waits = list(si.on_wait)
                    for w in waits[:-1]:
                        out.append(mybir.InstNoOp(
                            name=f"I-wf{n}", engine=inst.engine, ins=[],
                            outs=[],
                            sync_info=mybir.SyncInfo(on_wait=[w],
                                                     on_update=[])))
                        n += 1
                    si.on_wait = [waits[-1]]
                    changed = True
                out.append(inst)
            if changed:
                blk.instructions = out
    return n


def _build():
    if "nc" in _CACHE:
        return _CACHE["nc"]
    nc = bass.Bass("TRN2", target_bir_lowering=False, debug=False)

    hT_d = nc.dram_tensor("hT", [D, S], BF16, kind="ExternalInput").ap()
    wqkv_d = nc.dram_tensor("w_qkv", [D, NF * P], BF16, kind="ExternalInput").ap()
    wo_d = nc.dram_tensor("w_o", [4 * P, D], BF16, kind="ExternalInput").ap()
    cw_d = nc.dram_tensor("conv_w", [P, NF * 4], F32, kind="ExternalInput").ap()
    out_d = nc.dram_tensor("out", [S, D], BF16, kind="ExternalOutput").ap()

    hT_v = hT_d.rearrange("(ko ki) t -> ki ko t", ki=P)        # [128,16,2048]
    wqkv_v = wqkv_d.rearrange("(ko ki) f -> ki ko f", ki=P)    # [128,16,768]
    wo_v = wo_d.rearrange("(c ki) d -> ki c d", ki=P)          # [128,4,2048]
    out_v = out_d.rearrange("(po pi) d -> pi po d", pi=P)      # [128,16,2048]

    with tile.TileContext(nc) as tc, ExitStack() as ctx:
        const = ctx.enter_context(tc.tile_pool(name="const", bufs=1))
        p_ht = ctx.enter_context(tc.tile_pool(name="ht", bufs=2))
        p_work = ctx.enter_context(tc.tile_pool(name="work", bufs=2))
        p_probs = ctx.enter_context(tc.tile_pool(name="probs", bufs=3))
        p_ob = ctx.enter_context(tc.tile_pool(name="obp", bufs=4))
        ps_s = ctx.enter_context(tc.tile_pool(name="ps_s", bufs=2, space="PSUM"))
        ps_att = ctx.enter_context(tc.tile_pool(name="ps_att", bufs=1, space="PSUM"))
        ps_pr = ctx.enter_context(tc.tile_pool(name="ps_pr", bufs=2, space="PSUM"))

        # --- constants / persistent tensors ---
        # ones first: the PE warmup matmuls depend on it, and DVE executes
        # its queue in order
        ones_b = const.tile([P, P], BF16, tag="ones_b")
        nc.vector.memset(ones_b, 1.0)
        ident = const.tile([P, P], BF16, tag="ident")
        make_identity(nc, ident)
        # mask[k, x] = 1.0 if x >= k else 0.0 (shared by all diagonal tiles)
        mask = const.tile([P, 512], BF16, tag="mask")
        nc.gpsimd.memset(mask, 1.0)
        nc.gpsimd.affine_select(
            out=mask, in_=mask, pattern=[[1, 512]], base=0,
            channel_multiplier=-1, compare_op=mybir.AluOpType.is_ge, fill=0.0,
        )
        cw0 = const.tile([P, NF * 4], F32, tag="cw0")
        # (cw0's DMA and the cw/cwp copies are issued after the startup
        # wq/ht burst)
        cw = const.tile([P, NF * 4], F32, tag="cw")
        wq_sb = const.tile([P, KT, NF * P], BF16, tag="wq")
        wo_sb = const.tile([P, 4, D], BF16, tag="wo")
        # raw (pre-conv) qkv.T in fp32, with 3 leading zero columns so the
        # causal conv taps can read t-3..t-1 without edge cases
        qkvf = const.tile([P, NF, S + 4], F32, tag="qkvf")
        nc.scalar.memzero(qkvf[:, :, 0:4])
        qkvb = const.tile([P, NF, S], BF16, tag="qkvb")    # conv'd qkv.T (bf16)
        # v in [token, dh] with a ones column at 128: the PV matmul then
        # emits the softmax denominator as output column 128 for free
        # row stride 136 elements = 272 bytes: XBAR transpose destinations
        # must be 16-byte aligned (unaligned rows corrupt the transpose)
        vnat = const.tile([P, KT, 136], BF16, tag="vnat")
        nc.vector.memset(vnat[:, :, 128:129], 1.0)
        attT = const.tile([P, 4, S], BF16, tag="attT")     # attended.T per head

        def o_proj_chunk(qt, t4, dts=range(4), last=False):
            # output projection for token-128-tile t4 of q-tile qt
            tt16 = qt * 4 + t4
            ob = o_proj_chunk.ob
            if ob is None or o_proj_chunk.ob_tt != tt16:
                ob = p_ob.tile([P, 4, 512], BF16, tag="ob")
                o_proj_chunk.ob = ob
                o_proj_chunk.ob_tt = tt16
            for dt in dts:
                if last and (t4 * 4 + dt) % 2 == 1:
                    # the scores PSUM pool is free during the final chunks;
                    # alternating pools doubles the rotation depth
                    op = ps_s.tile([P, 512], F32, tag="s", name="op2")
                else:
                    op = ps_pr.tile([P, 512], F32, tag="proj")
                for fc4 in range(4):
                    nc.tensor.matmul(
                        op, lhsT=attT[:, fc4, ds(tt16 * P, P)],
                        rhs=wo_sb[:, fc4, ds(dt * 512, 512)],
                        start=(fc4 == 0), stop=(fc4 == 3))
                # PSUM -> bf16 SBUF copy: DVE during attention phases
                # (ACT is exp-bound); ACT in the final stretch where it is
                # otherwise idle
                if last:
                    nc.scalar.copy(ob[:, dt, :], op)
                else:
                    nc.vector.tensor_copy(ob[:, dt, :], op)
                if last:
                    eng = nc.sync if dt % 2 == 0 else nc.scalar
                    eng.dma_start(out_v[:, tt16, ds(dt * 512, 512)],
                                  ob[:, dt, :])
            if not last and dts[-1] == 3:
                nc.sync.dma_start(out_v[:, tt16, :], ob)

        o_proj_chunk.ob = None
        o_proj_chunk.ob_tt = -1

        fins = []

        def flush_fins():
            while fins:
                fins.pop(0)()

        def attn_head(qt, h):
            nk = 4 * (qt + 1)
            lasthead = (qt == NQT - 1 and h == 3)
            # diagonal k-tiles first (their conv dependencies are a full
            # phase old under the staggered schedule), then full tiles
            units = [("diag", [4 * qt + j]) for j in range(4)]
            units += [("full", [kt]) for kt in range(4 * qt)]
            nu = len(units)
            inj = {}
            if qt > 0:
                if qt < NQT - 1:
                    # the previous q-tile's h3 attT writes land late in the
                    # previous pair; skip h0 so nothing stalls on them
                    work = {0: [], 1: [(0, 0), (0, 1), (0, 2), (0, 3)],
                            2: [(1, 0), (1, 1), (1, 2), (1, 3)],
                            3: [(2, 0), (2, 1), (2, 2), (2, 3),
                                (3, 0), (3, 1), (3, 2), (3, 3)]}[h]
                else:
                    work = [(h, 0), (h, 1), (h, 2), (h, 3)]
                if work:
                    step = (nu - 2) / len(work)
                    for i, w in enumerate(work):
                        inj.setdefault(2 + int(i * step), []).append(w)
            # one PSUM bank per q-subtile accumulator: concurrent matmul
            # accumulation groups must not share a bank (hardware
            # start/has_written tracking is bank-granular). Allocated lazily
            # at the first flush so the previous head's deferred fin tiles
            # rotate in front of them.
            sub_t = []

            def get_sub():
                if not sub_t:
                    sub_t.extend(
                        ps_att.tile([P, 129], F32, tag=f"att{s}", name="attx")
                        for s in range(4))
                return sub_t

            scount = [0, 0, 0, 0]
            stotal = [4 * qt + s + 1 for s in range(4)]
            state = dict(prev=None, prev2=None)
            pr_quad = None

            def flush(unit):
                for (pr, x0, kt) in unit:
                    for s in range(x0 // P, 4):
                        nc.tensor.matmul(
                            get_sub()[s], lhsT=pr[:, ds(s * P, P)],
                            rhs=vnat[:, kt, 0:129],
                            start=(scount[s] == 0),
                            stop=(scount[s] == stotal[s] - 1))
                        scount[s] += 1

            for ui, (kind, kts) in enumerate(units):
                if ui == 1:
                    flush_fins()
                for (t4i, dti) in inj.get(ui, ()):
                    o_proj_chunk(qt - 1, t4i, dts=[dti])
                if kts[0] % 4 == 0:
                    pr_quad = p_probs.tile([P, 4, 512], BF16, tag="probs", bufs=4)
                kt = kts[0]
                if kind == "full":
                    sp = ps_s.tile([P, 512], F32, tag="s")
                    nc.tensor.matmul(
                        sp, lhsT=qkvb[:, 4, ds(kt * P, P)],
                        rhs=qkvb[:, h, ts(qt, 512)],
                        start=True, stop=True)
                    pr = pr_quad[:, kt % 4, :]
                    nc.scalar.activation(pr, sp, EXP, scale=ISQ)
                    cur = [(pr, 0, kt)]
                else:
                    j = kt - 4 * qt
                    x0 = j * P
                    F = 512 - x0
                    sp = ps_s.tile([P, 512], F32, tag="s")
                    nc.tensor.matmul(
                        sp[:, x0:512],
                        lhsT=qkvb[:, 4, ds(kt * P, P)],
                        rhs=qkvb[:, h, ds(qt * 512 + x0, F)],
                        start=True, stop=True)
                    pr = pr_quad[:, kt % 4, :]
                    nc.scalar.activation(
                        pr[:, x0:512], sp[:, x0:512], EXP, scale=ISQ)
                    nc.vector.tensor_mul(
                        pr[:, x0:512], pr[:, x0:512], mask[:, 0:F])
                    cur = [(pr, x0, kt)]
                if state["prev2"] is not None:
                    flush(state["prev2"])
                state["prev2"] = state["prev"]
                state["prev"] = cur
            if state["prev2"] is not None:
                flush(state["prev2"])
            if state["prev"] is not None:
                flush(state["prev"])

            # ---- finalize: reciprocal of the denominator columns, then
            # per-partition scale into bf16 [q, dh], then PE transpose into
            # attT. Phased (scales, transposes, copies) and deferred into
            # the next head unless this is the very last one.
            def fin():
                rec = p_work.tile([P, 4], F32, tag="rec4")
                for s in range(4):
                    nc.vector.reciprocal(rec[:, s:s + 1],
                                         sub_t[s][:, 128:129])
                anorms = []
                for s in range(4):
                    anorm = p_work.tile([P, P], BF16, tag=f"anorm{s}",
                                        name="anorm", bufs=3)
                    nc.vector.tensor_scalar_mul(
                        anorm, sub_t[s][:, 0:P], rec[:, s:s + 1])
                    anorms.append(anorm)
                tps = []
                for s in range(4):
                    tp = ps_att.tile([P, P], BF16, tag=f"att{s}", name="tp")
                    nc.tensor.transpose(tp, anorms[s], ident)
                    tps.append(tp)
                for s in range(4):
                    nc.vector.tensor_copy(
                        attT[:, h, ds(qt * 512 + s * P, P)], tps[s])
            if lasthead:
                fin()
            else:
                fins.append(fin)

        def attn_heads(qt):
            for h in range(4):
                attn_head(qt, h)
                yield

        def conv(fc, eng, cwt, etag):
            # conv taps: out[t] = x[t] + sum_k x[t+k-3]*w[k]
            t0 = conv.t0
            tmp = p_work.tile([P, 512], F32, tag="ctmp" + etag, name="ctmp")
            eng.scalar_tensor_tensor(
                tmp, qkvf[:, fc, ds(t0 + 0, 512)],
                cwt[:, fc * 4 + 0: fc * 4 + 1],
                qkvf[:, fc, ds(t0 + 3, 512)], op0=MULT, op1=ADD)
            eng.scalar_tensor_tensor(
                tmp, qkvf[:, fc, ds(t0 + 1, 512)],
                cwt[:, fc * 4 + 1: fc * 4 + 2], tmp, op0=MULT, op1=ADD)
            eng.scalar_tensor_tensor(
                tmp, qkvf[:, fc, ds(t0 + 2, 512)],
                cwt[:, fc * 4 + 2: fc * 4 + 3], tmp, op0=MULT, op1=ADD)
            eng.scalar_tensor_tensor(
                qkvb[:, fc, ds(t0, 512)], qkvf[:, fc, ds(t0 + 3, 512)],
                cwt[:, fc * 4 + 3: fc * 4 + 4], tmp, op0=MULT, op1=ADD)

        # conv engine assignment: q-head chains 0,1 on DVE (light during
        # projection), the rest on Pool
        CONV_ENG = {}

        def run_conv(fc):
            eng, cwt, etag = CONV_ENG[fc]
            conv(fc, eng, cwt, etag)

        # ------- Staggered pipeline: proj(tt) runs one phase AHEAD of
        # attn(tt-1), so attention never waits on freshly conv'd qkv and the
        # PE queue (strict FIFO) always has ready matmuls at phase edges.
        CONV_ENG.update({fc: (nc.vector, cw, "v") for fc in range(NF)})
        FC_ORDER = [0, 4, 5, 1, 2, 3]

        # warm up the PE p-state during the initial DMA wait with dummy
        # matmuls into a scratch PSUM tile (output never read)
        warm = ps_pr.tile([P, 512], F32, tag="proj", name="warm")
        for _ in range(12):
            nc.tensor.matmul(warm[:, 0:128], lhsT=ones_b, rhs=ones_b,
                             start=True, stop=True, skip_group_check=True)

        ht_cur = p_ht.tile([P, KT, 512], BF16, tag="ht")
        # fine-grained startup: single wq chunks + ht pieces ordered so the
        # first contraction chunks land first
        nc.sync.dma_start(ht_cur[:, 0:1, :], hT_v[:, 0:1, ts(0, 512)])
        nc.sync.dma_start(wq_sb[:, 0:1, 0:384], wqkv_v[:, 0:1, 0:384])
        nc.sync.dma_start(wq_sb[:, 0:1, 384:768], wqkv_v[:, 0:1, 384:768])
        nc.sync.dma_start(ht_cur[:, 1:2, :], hT_v[:, 1:2, ts(0, 512)])
        nc.sync.dma_start(wq_sb[:, 1:2, :], wqkv_v[:, 1:2, :])
        for k2 in range(1, 8):
            nc.sync.dma_start(ht_cur[:, ds(k2 * 2, 2), :],
                              hT_v[:, ds(k2 * 2, 2), ts(0, 512)])
            nc.sync.dma_start(wq_sb[:, ds(2 * k2, 2), :],
                              wqkv_v[:, ds(2 * k2, 2), :])
        nc.sync.dma_start(cw0, cw_d)
        # conv ops read cw via per-engine copies so their DMA wait lands
        # here, not on the (wait-slot-limited) scalar_tensor_tensor chains
        nc.vector.tensor_copy(cw, cw0)

        def emit_vnat(tt):
            # v (fc=5) of this token tile -> natural [token, dh] layout via
            # PE transpose (the XBAR transpose path raced under load)
            for j in range(4):
                kt_i = tt * 4 + j
                tv = ps_pr.tile([P, P], BF16, tag="proj", name="tv")
                nc.tensor.transpose(tv, qkvb[:, 5, ds(kt_i * P, P)], ident)
                nc.vector.tensor_copy(vnat[:, kt_i, 0:P], tv)

        def proj_blocks(tt, ht):
            # generator: yields after each fc block so attention heads of the
            # previous q-tile can interleave as PE filler
            conv.t0 = t0 = tt * 512
            if tt == 0:
                # first half contraction-outer (6 concurrent PSUM groups, so
                # matmuls start as soon as the first chunk pair lands); second
                # half fc-outer so each fc's copy+conv chain starts early
                gA = ps_s.tile([P, 512], F32, tag="s", name="gA")
                gB = ps_s.tile([P, 512], F32, tag="s", name="gB")
                gl = [ps_att.tile([P, 512], F32, tag=f"att{s}", name="gx")
                      for s in range(4)]
                groups = {0: gA, 4: gB, 5: gl[0],
                          1: gl[1], 2: gl[2], 3: gl[3]}
                for kk in range(KT // 2):
                    for fc in FC_ORDER:
                        nc.tensor.matmul(
                            groups[fc], lhsT=wq_sb[:, kk, ds(fc * P, P)],
                            rhs=ht[:, kk, :],
                            start=(kk == 0), stop=False)
                for fc in FC_ORDER:
                    for kk in range(KT // 2, KT):
                        nc.tensor.matmul(
                            groups[fc], lhsT=wq_sb[:, kk, ds(fc * P, P)],
                            rhs=ht[:, kk, :],
                            start=False, stop=(kk == KT - 1))
                    nc.scalar.copy(qkvf[:, fc, ds(3 + t0, 512)], groups[fc])
                for fc in FC_ORDER:
                    if fc in (4, 5):
                        run_conv(fc)
                    if fc == 5:
                        emit_vnat(tt)
                for fc in (0, 1, 2, 3):
                    run_conv(fc)
            else:
                for fc in FC_ORDER:
                    pp = ps_pr.tile([P, 512], F32, tag="proj", name="pp")
                    for kk in range(KT):
                        nc.tensor.matmul(
                            pp, lhsT=wq_sb[:, kk, ds(fc * P, P)],
                            rhs=ht[:, kk, :],
                            start=(kk == 0), stop=(kk == KT - 1),
                        )
                    nc.scalar.copy(qkvf[:, fc, ds(3 + t0, 512)], pp)
                    # k/v convs run inline (their consumers are earliest in
                    # the next phase); q convs are emitted at phase end so
                    # attention's DVE/Pool work isn't queued behind them
                    if fc in (4, 5):
                        run_conv(fc)
                    if fc == 5:
                        emit_vnat(tt)
                    yield
                for fc in (0, 1, 2, 3):
                    run_conv(fc)

        def drain(gen):
            if gen is not None:
                for _ in gen:
                    pass

        for tt in range(NTT):
            if tt + 1 < NTT:
                ht_next = p_ht.tile([P, KT, 512], BF16, tag="ht")
            else:
                ht_next = None
            prefetched = False

            def emit_prefetch(tt, ht_next):
                for k4 in range(4):
                    nc.sync.dma_start(
                        ht_next[:, ds(k4 * 4, 4), :],
                        hT_v[:, ds(k4 * 4, 4), ts(tt + 1, 512)])
                if tt == 0:
                    # w_o load deferred out of the startup DMA burst, but
                    # before the attention-phase transposes start competing
                    nc.sync.dma_start(wo_sb, wo_v)

            pg = proj_blocks(tt, ht_cur)
            ag = attn_heads(tt - 1) if tt >= 1 else None
            if tt == 1:
                # attn(0) depends on tt=0's conv chains, which retire during
                # proj(1); give proj one block's head start
                next(pg)
            # alternate: fc projection block, then attention head
            step = 0
            while True:
                stopped = True
                try:
                    next(pg)
                    stopped = False
                except StopIteration:
                    pass
                if ag is not None:
                    try:
                        next(ag)
                        stopped = False
                    except StopIteration:
                        ag = None
                step += 1
                if step == 4 and ht_next is not None:
                    # prefetch next token tile's hidden states mid-pair so
                    # the attention fins' attT transposes win HWDGE first
                    emit_prefetch(tt, ht_next)
                    prefetched = True
                if stopped and ag is None:
                    break
            if ht_next is not None and not prefetched:
                emit_prefetch(tt, ht_next)
            ht_cur = ht_next
        drain(attn_heads(NQT - 1))
        flush_fins()
        for t4 in range(4):
            # the last two tiles use per-dt DMAs so no big coalesced
            # transfer sits in front of the final small ones
            o_proj_chunk(NQT - 1, t4, last=(t4 == 3))

    _legalize_waits(nc)
    _CACHE["nc"] = nc
    return nc


def _prep_inputs(hidden_states, w_q, w_k, w_v, w_o, conv_w):
    """Build the 8 per-core input maps (host-side shard + bf16 cast)."""
    bf = ml_dtypes.bfloat16
    in_maps = []
    for c in range(8):
        b, g = c // 4, c % 4
        hT = np.ascontiguousarray(hidden_states[b].T).astype(bf)
        wqkv = np.concatenate(
            [w_q[:, g * 512:(g + 1) * 512],
             w_k[:, g * 128:(g + 1) * 128],
             w_v[:, g * 128:(g + 1) * 128]], axis=1).astype(bf)
        wo = np.ascontiguousarray(w_o[g * 512:(g + 1) * 512, :]).astype(bf)
        cw = np.concatenate(
            [conv_w[g * 512:(g + 1) * 512],
             conv_w[2048 + g * 128: 2048 + (g + 1) * 128],
             conv_w[2560 + g * 128: 2560 + (g + 1) * 128]], axis=0)  # [768,4]
        cw = np.ascontiguousarray(
            cw.reshape(NF, P, 4).transpose(1, 0, 2).reshape(P, NF * 4)
        ).astype(np.float32)
        in_maps.append({"hT": hT, "w_qkv": wqkv, "w_o": wo, "conv_w": cw})
    return in_maps


def kernel(hidden_states, w_q, w_k, w_v, w_o, conv_w, _trace=False):
    nc = _build()
    in_maps = _prep_inputs(
        np.asarray(hidden_states, dtype=np.float32),
        np.asarray(w_q, dtype=np.float32),
        np.asarray(w_k, dtype=np.float32),
        np.asarray(w_v, dtype=np.float32),
        np.asarray(w_o, dtype=np.float32),
        np.asarray(conv_w, dtype=np.float32),
    )
    res = run_bass_kernel_spmd(nc, in_maps, core_ids=list(range(8)),
                               trace=_trace)
    outs = [np.asarray(r["out"], dtype=np.float32) for r in res.results]
    full = np.empty((2, S, D), dtype=np.float32)
    for b in range(2):
        full[b] = outs[4 * b] + outs[4 * b + 1] + outs[4 * b + 2] + outs[4 * b + 3]
    if _trace:
        kernel.last_results = res
    return full
